# revision 1
# baseline (speedup 1.0000x reference)
"""Single-head attention with per-sample padding masks, data-parallel over
batch across 8 Trainium2 NeuronCores (one batch element per core).

kernel(**inputs) takes the FULL unsharded inputs (as produced by the
problem's setup_inputs) and returns the FULL [B, N, D] float32 output.

Device program per core (SPMD, no collectives), S^T ("transposed scores")
formulation with residual-compensated fp8 matmuls:

  Every fp8 DoubleRow matmul runs at 0.5 PE cycles/column (2x bf16).
  A bf16-accuracy product a@b is computed as three fp8 terms
      a8@b8 + a8@br + ar@b8        (a8=fp8(a), ar=fp8(a-a8))
  fp8xfp8 products are exact in the fp32 PSUM, so the only error is the
  dropped ar@br term (~0.4% -- bf16 level) at 1.5 cycles/col vs bf16's 2.

  q' = Wq'.T @ x + WSCALE*bq      [E, N]  (W' = WSCALE*W staged as fp8 pair;
  k' = likewise;  v' = x.T @ Wv'  [N, D]   the WSCALE factor is never divided
                                           out -- it cancels in the exp scale
                                           and the staged output constants)
  q8/qr, k8/kr quantized on-device (ACT + DVE), v' stays bf16 + fp8 pair.
  ST[j, i] = k'.T @ q'            2-term fp8 DR (k8q8 + k8qr; the dropped
                                  kr@q8 term costs ~8.6e-3 rel err in
                                  quadrature, measured 8.9e-3 total vs the
                                  2e-2 gate), [128 j, 512 i] blocks
  AT[j, i] = exp(s/WSCALE^2 * ST + maskb_j)  maskb_j = 0 valid / -1e9 padded
                                  key -> exp -> 0. Mask rides the ACT bias.
  out[i, :] = (AT.T @ v) * valid_i/rowsum_i + colsum(v)/N * (1-valid_i)
      rowsum_i = AT.T @ ones (PE);  padded queries get mean(v) over all
      N rows, matching the reference's all-masked-row softmax.
"""

import math
import sys
from contextlib import ExitStack

import numpy as np

sys.path.insert(0, "/opt/trn_rl_repo")

import concourse.mybir as mybir  # noqa: E402
import concourse.tile as tile  # noqa: E402
from concourse import bacc  # noqa: E402

P = 128
B, N, D = 8, 2048, 512
FB = 512  # psum free-dim block (one bank)
MASK_VAL = -1.0e9
# Weights pre-scaled into fp8 normal range.  32 (not 64): q' = WSCALE*q must
# stay below fp8 e4m3 max 240 -- |q| ~ N(0, 0.58), 240/32 = 7.5 is ~11 sigma,
# while 240/64 = 3.75 is reachable and one overflow -> inf -> NaN rows.
WSCALE = 32.0


def build_attention_nc(n=N, d=D, debug=False):
    """Build the one-core Bass program. Returns the compiled Bacc module."""
    f32 = mybir.dt.float32
    bf16 = mybir.dt.bfloat16
    fp8 = mybir.dt.float8e4
    DR = mybir.MatmulPerfMode.DoubleRow
    ec_n = d // P  # embedding chunks (contraction over E and D)
    nt = n // P  # 128-row seq tiles (key tiles jt / query chunks it)
    nb = n // FB  # 512-col seq blocks (query blocks ib)
    s = 1.0 / math.sqrt(d)

    nc = bacc.Bacc(None, target_bir_lowering=False, debug=debug)

    # inputs staged host-side in [P, chunk, cols] layout so every DMA reads
    # contiguous bytes per partition (the on-device (c p)->p c rearrange
    # costs ~5x in strided descriptors)
    x8_d = nc.declare_dram_parameter("x8", [P, ec_n, n], fp8, isOutput=False)
    xr_d = nc.declare_dram_parameter("xr", [P, ec_n, n], fp8, isOutput=False)
    w8_ds, wr_ds = {}, {}
    for wn in ("wq", "wk", "wv"):
        w8_ds[wn] = nc.declare_dram_parameter(
            wn + "8", [P, ec_n, d], fp8, isOutput=False
        )
        wr_ds[wn] = nc.declare_dram_parameter(
            wn + "r", [P, ec_n, d], fp8, isOutput=False
        )
    bq_d = nc.declare_dram_parameter("bq", [P, ec_n], f32, isOutput=False)
    bk_d = nc.declare_dram_parameter("bk", [P, ec_n], f32, isOutput=False)
    maskb_d = nc.declare_dram_parameter("maskb", [P, nt], f32, isOutput=False)
    avalid_d = nc.declare_dram_parameter("avalid", [P, nt], f32, isOutput=False)
    bsel_d = nc.declare_dram_parameter("bsel", [P, nt], f32, isOutput=False)
    out_d = nc.declare_dram_parameter("out", [n, d], f32, isOutput=True)

    Ident = mybir.ActivationFunctionType.Identity
    Exp = mybir.ActivationFunctionType.Exp
    Add = mybir.AluOpType.add
    Mult = mybir.AluOpType.mult
    Sub = mybir.AluOpType.subtract

    with tile.TileContext(nc) as tc, ExitStack() as ctx:
        const = ctx.enter_context(tc.tile_pool(name="const", bufs=1))
        big = ctx.enter_context(tc.tile_pool(name="big", bufs=1))
        work = ctx.enter_context(tc.tile_pool(name="work", bufs=8))
        small = ctx.enter_context(tc.tile_pool(name="small", bufs=4))
        psum_s = ctx.enter_context(tc.tile_pool(name="psum_s", bufs=4, space="PSUM"))
        psum_av = ctx.enter_context(tc.tile_pool(name="psum_av", bufs=3, space="PSUM"))
        psum_rs = ctx.enter_context(tc.tile_pool(name="psum_rs", bufs=1, space="PSUM"))

        # ---- constants / parameters into SBUF ----
        ones_col = const.tile([P, 1], bf16)
        nc.vector.memset(ones_col, 1.0)
        ones_row = const.tile([1, P], bf16)
        nc.vector.memset(ones_row, 1.0)
        bq_sb = const.tile([P, ec_n], f32)
        bk_sb = const.tile([P, ec_n], f32)
        maskb_sb = const.tile([P, nt], f32)
        avalid_sb = const.tile([P, nt], f32)
        bsel_sb = const.tile([P, nt], f32)

        x8_sb = big.tile([P, ec_n, n], fp8)
        xr_sb = big.tile([P, ec_n, n], fp8)
        w8_sb = {wn: big.tile([P, ec_n, d], fp8, name=wn + "8_sb") for wn in ("wq", "wk", "wv")}
        wr_sb = {wn: big.tile([P, ec_n, d], fp8, name=wn + "r_sb") for wn in ("wq", "wk", "wv")}
        # coalesced contiguous DMAs (one per tensor / column-block), spread
        # over the 3 DMA-capable queues so the first unit starts after ~1us.
        # 3-way queue parallelism beats HWDGE-only: the gpsimd (SWDGE) issues
        # cost ~2.1us each of Pool time, but the Pool is idle this early and
        # the extra queue keeps x8/xr/weights streaming concurrently
        nc.scalar.dma_start(out=w8_sb["wq"], in_=w8_ds["wq"][:, :, :])
        nc.scalar.dma_start(out=wr_sb["wq"], in_=wr_ds["wq"][:, :, :])
        for ib in range(nb):
            if ib == 0:
                # first block split in dc halves: the opening DoubleRow pair
                # only needs dc 0..1, so the PE starts earlier
                for h in range(2):
                    nc.sync.dma_start(
                        out=x8_sb[:, 2 * h : 2 * h + 2, 0:FB],
                        in_=x8_d[:, 2 * h : 2 * h + 2, 0:FB],
                    )
                    nc.gpsimd.dma_start(
                        out=xr_sb[:, 2 * h : 2 * h + 2, 0:FB],
                        in_=xr_d[:, 2 * h : 2 * h + 2, 0:FB],
                    )
            else:
                nc.sync.dma_start(
                    out=x8_sb[:, :, ib * FB : (ib + 1) * FB],
                    in_=x8_d[:, :, ib * FB : (ib + 1) * FB],
                )
                nc.gpsimd.dma_start(
                    out=xr_sb[:, :, ib * FB : (ib + 1) * FB],
                    in_=xr_d[:, :, ib * FB : (ib + 1) * FB],
                )
            if ib == 0:
                # wk on the sync queue: it lands right when K(0) needs it,
                # instead of queueing behind wq on the scalar queue
                nc.sync.dma_start(out=w8_sb["wk"], in_=w8_ds["wk"][:, :, :])
                nc.sync.dma_start(out=wr_sb["wk"], in_=wr_ds["wk"][:, :, :])
                nc.gpsimd.dma_start(out=bq_sb, in_=bq_d[:, :])
                nc.gpsimd.dma_start(out=bk_sb, in_=bk_d[:, :])
            elif ib == 1:
                nc.scalar.dma_start(out=w8_sb["wv"], in_=w8_ds["wv"][:, :, :])
                nc.scalar.dma_start(out=wr_sb["wv"], in_=wr_ds["wv"][:, :, :])
                nc.gpsimd.dma_start(out=maskb_sb, in_=maskb_d[:, :])
            elif ib == 2:
                nc.gpsimd.dma_start(out=avalid_sb, in_=avalid_d[:, :])
                nc.gpsimd.dma_start(out=bsel_sb, in_=bsel_d[:, :])

        q8_sb = big.tile([P, ec_n, n], fp8)
        qr_sb = big.tile([P, ec_n, n], fp8)
        k8_sb = big.tile([P, ec_n, n], fp8)
        kr_sb = big.tile([P, ec_n, n], fp8)
        v_sb = big.tile([P, nt, d], bf16)
        v8_sb = big.tile([P, nt, d], fp8)
        vr_sb = big.tile([P, nt, d], fp8)
        a8_sb = big.tile([P, nt, n], fp8)
        ar_sb = big.tile([P, nt, n], fp8)
        mrep_sb = big.tile([P, d], f32)
        msum_row = big.tile([1, d], bf16)
        ones2_col = const.tile([P, 2, 1], fp8)
        nc.vector.memset(ones2_col, 1.0)
        # p-state warm-up: keep the PE busy on throwaway matmuls during the
        # initial DMA window so real work starts at full clock (the PE drops
        # to 1.2GHz for 3us after any idle period)
        warm = const.tile([1, FB], bf16)
        nc.vector.memset(warm, 1.0)
        for _ in range(6):
            pd = psum_rs.tile([P, FB], f32, tag="prs")
            nc.tensor.matmul(pd, lhsT=ones_row, rhs=warm, start=True, stop=True)

        def mm3(ps, a8, ar, b8, br, asl, bsl, drop_ar=False, drop_br=False):
            """psum += a@b as a8@b8 + a8@br + ar@b8 (fp8 DoubleRow terms).
            asl/bsl: lambdas slicing [P, ec-pair, cols] views.
            drop_ar: omit ar@b8 -- scores (dropped k-residual measured
            +8.6e-3 rel err in quadrature, buys 13.7us PE).
            drop_br: omit a8@br -- Q-projection x-residual (gaussian, same
            statistics as the scores drop; W-residuals are NOT droppable:
            uniform weights quantize 1.6x worse, measured)."""
            terms = [(a8, b8)]
            if not drop_br:
                terms.append((a8, br))
            if not drop_ar:
                terms.append((ar, b8))
            nmm = len(terms) * (ec_n // 2)
            i = 0
            for ta, tb in terms:
                for ecp in range(0, ec_n, 2):
                    nc.tensor.matmul(
                        ps,
                        lhsT=asl(ta, ecp),
                        rhs=bsl(tb, ecp),
                        start=(i == 0),
                        stop=(i == nmm - 1),
                        perf_mode=DR,
                    )
                    i += 1

        # ---- Q/K projections -> fp8 pairs; V projection -> bf16 ----
        # The WSCALE on W is never divided out on-device: q', k', v' carry a
        # 64x factor that cancels in the exp scale (s/WSCALE^2) and in the
        # output normalization (avalid/bsel staged pre-divided by WSCALE).
        # q8 (ACT): fp8(ps + 64 bq) straight from PSUM; qr (DVE): one
        # scalar_tensor_tensor (ps + 64 bq) - q8.
        def emit_qkproj(ib):
            for wn, b_sb, o8, orr in (
                ("wq", bq_sb, q8_sb, qr_sb),
                ("wk", bk_sb, k8_sb, kr_sb),
            ):
                for ec in range(ec_n):
                    # alternate PSUM pools (psum_av idles during projections)
                    # for a deeper effective rotation against consumer lag
                    if ec % 2 == 0:
                        ps = psum_s.tile([P, FB], f32, tag="ps", name="ps_qk")
                    else:
                        ps = psum_av.tile([P, FB], f32, tag="pav", name="ps_qk")
                    mm3(
                        ps,
                        w8_sb[wn],
                        wr_sb[wn],
                        x8_sb,
                        xr_sb,
                        lambda t, ecp, ec=ec: t[:, ecp : ecp + 2, ec * P : (ec + 1) * P],
                        lambda t, ecp, ib=ib: t[:, ecp : ecp + 2, ib * FB : (ib + 1) * FB],
                        # NOTE: projections keep all 3 terms.  Measured:
                        # dropping the W-residual -> 1.7e-2 rel err (uniform
                        # W quantizes 1.6x worse than gaussian); dropping the
                        # Q-side x-residual -> 1.8e-2 (the dropped error
                        # correlates with k' through the shared x, biasing
                        # self-scores).  Only the post-projection scores
                        # residual (kr@q8) is cheap to drop.
                        drop_br=False,
                    )
                    sl = (slice(None), ec, slice(ib * FB, (ib + 1) * FB))
                    nc.scalar.activation(
                        o8[sl], ps, Ident, bias=b_sb[:, ec : ec + 1], scale=1.0
                    )
                    nc.vector.scalar_tensor_tensor(
                        out=orr[sl],
                        in0=ps,
                        scalar=b_sb[:, ec : ec + 1],
                        in1=o8[sl],
                        op0=Add,
                        op1=Sub,
                    )

        def emit_vproj(jt):
            ps = psum_s.tile([P, d], f32, tag="ps")
            mm3(
                ps,
                x8_sb,
                xr_sb,
                w8_sb["wv"],
                wr_sb["wv"],
                lambda t, ecp, jt=jt: t[:, ecp : ecp + 2, jt * P : (jt + 1) * P],
                lambda t, ecp: t[:, ecp : ecp + 2, :],
            )
            nc.scalar.activation(v_sb[:, jt, :], ps, Ident, bias=0.0, scale=1.0)
            nc.gpsimd.tensor_copy(v8_sb[:, jt, :], v_sb[:, jt, :])
            nc.vector.tensor_tensor(
                out=vr_sb[:, jt, :], in0=v_sb[:, jt, :], in1=v8_sb[:, jt, :], op=Sub
            )

        # (projection emission happens in the combined driver below, where
        # scores(0) units are interleaved as soon as their k-columns exist)

        # ---- column-sum of v (for padded-query rows: mean = colsum/N);
        # one matmul per scores(0) unit; fp8 DoubleRow is rejected here by
        # walrus (s3_lw_dual_fp8_restrictions: M=1 stationary operand), so
        # this stays a bf16 matmul over v ----
        mean_state = {}

        def emit_mean_colsum_mm(jt):
            if jt == 0:
                mean_state["pm"] = psum_rs.tile(
                    [1, d], f32, tag="prs", name="pm_mean"
                )
            nc.tensor.matmul(
                mean_state["pm"],
                lhsT=ones_col,
                rhs=v_sb[:, jt, :],
                start=(jt == 0),
                stop=(jt == nt - 1),
            )

        def emit_mean_rep():
            pr = psum_rs.tile([P, d], f32, tag="prs")
            nc.tensor.matmul(pr, lhsT=ones_row, rhs=msum_row, start=True, stop=True)
            nc.scalar.activation(mrep_sb, pr, Ident, bias=0.0, scale=1.0)

        # ---- attention ----
        scores_state = {}

        def emit_scores_unit(ib, jt):
            ps = psum_s.tile([P, FB], f32, tag="ps")
            mm3(
                ps,
                k8_sb,
                kr_sb,
                q8_sb,
                qr_sb,
                lambda t, ecp, jt=jt: t[:, ecp : ecp + 2, jt * P : (jt + 1) * P],
                lambda t, ecp, ib=ib: t[:, ecp : ecp + 2, ib * FB : (ib + 1) * FB],
                drop_ar=True,
            )
            ab = work.tile([P, FB], bf16)
            nc.scalar.activation(
                ab,
                ps,
                Exp,
                bias=maskb_sb[:, jt : jt + 1],
                scale=s / (WSCALE * WSCALE),
            )
            sl = (slice(None), jt, slice(ib * FB, (ib + 1) * FB))
            nc.gpsimd.tensor_copy(a8_sb[sl], ab)
            nc.vector.tensor_tensor(out=ar_sb[sl], in0=ab, in1=a8_sb[sl], op=Sub)

        def emit_scores(ib, av_of=None, mean=False):
            # av_of: interleave the 4 AV chunks of that (older, data-ready)
            # block between this block's scores units, so a PE stall on one
            # pipeline is filled by work from the other.  mean: interleave
            # the mean-colsum matmuls (scores(0) has no AV work yet and the
            # 2-term scores PE outpaces the exp, so it starves otherwise)
            for jt in range(nt):
                if av_of is not None and jt >= 4 and (jt - 4) % 4 == 0:
                    emit_av_chunk(av_of * 4 + (jt - 4) // 4)
                emit_scores_unit(ib, jt)
                if mean:
                    emit_mean_colsum_mm(jt)
            if av_of is not None:
                emit_av_chunk(av_of * 4 + 3)
            if mean:
                nc.vector.tensor_copy(msum_row, mean_state["pm"])

        def emit_av_chunk(it):
            pav = psum_av.tile([P, d], f32, tag="pav")
            prs = psum_rs.tile([P, 1], f32, tag="prs")
            av_terms = [(a8_sb, v8_sb), (a8_sb, vr_sb), (ar_sb, v8_sb)]
            nmm = len(av_terms) * (nt // 2)
            i = 0
            for ta, tv in av_terms:
                for jtp in range(0, nt, 2):
                    nc.tensor.matmul(
                        pav,
                        lhsT=ta[:, jtp : jtp + 2, it * P : (it + 1) * P],
                        rhs=tv[:, jtp : jtp + 2, :],
                        start=(i == 0),
                        stop=(i == nmm - 1),
                        perf_mode=DR,
                    )
                    i += 1
            i = 0
            for ta in (a8_sb, ar_sb):
                for jtp in range(0, nt, 2):
                    nc.tensor.matmul(
                        prs,
                        lhsT=ta[:, jtp : jtp + 2, it * P : (it + 1) * P],
                        rhs=ones2_col,
                        start=(i == 0),
                        stop=(i == 2 * (nt // 2) - 1),
                        perf_mode=DR,
                    )
                    i += 1
            rinv = small.tile([P, 1], f32)
            nc.vector.reciprocal(rinv, prs)
            a_eff = small.tile([P, 1], f32)
            nc.vector.tensor_mul(a_eff, rinv, avalid_sb[:, it : it + 1])
            tmp2 = work.tile([P, d], f32)
            nc.scalar.activation(
                tmp2, mrep_sb, Ident, bias=0.0, scale=bsel_sb[:, it : it + 1]
            )
            outt = work.tile([P, d], f32)
            nc.vector.scalar_tensor_tensor(
                out=outt,
                in0=pav,
                scalar=a_eff,
                in1=tmp2,
                op0=Mult,
                op1=Add,
            )
            nc.sync.dma_start(out=out_d[it * P : (it + 1) * P, :], in_=outt)

        # software-pipelined emission:
        #  - Q/K (DVE-heavy) and V (Pool-heavy) projection groups interleave;
        #  - scores(0) units slot into the projection phase one group behind
        #    their k-columns, giving the PE filler work for consumer stalls;
        #  - AV chunks of block ib interleave into scores(ib+2) where their
        #    a8/ar inputs are long ready;
        #  - the mean chain hides inside the scores blocks.
        # V(12-15) moves out of the projection phase into scores(0): the
        # 2-term scores PE outpaces the ACT exp there and starves without
        # filler; the colsum block follows once all of v exists
        for ib in range(nb):
            emit_qkproj(ib)
            for jt in range(ib * 4, ib * 4 + 4):
                if jt < 12:
                    emit_vproj(jt)
        # mean colsum as two CLOSED 8-matmul sub-groups interleaved into
        # scores(0) (one open 16-mm group here measured +5us), halves summed
        # on the DVE within one partition
        msum_h = big.tile([1, 2, d], f32, name="msum_h")

        def emit_colsum_half(h):
            pmh = psum_rs.tile([1, d], f32, tag="prs", name="pm_h")
            for k in range(8):
                nc.tensor.matmul(
                    pmh,
                    lhsT=ones_col,
                    rhs=v_sb[:, 8 * h + k, :],
                    start=(k == 0),
                    stop=(k == 7),
                )
            nc.vector.tensor_copy(msum_h[:, h, :], pmh)

        for jt in range(nt):
            if jt in (3, 6, 9, 12):
                emit_vproj(jt // 3 + 11)
            emit_scores_unit(0, jt)
            if jt == 7:
                emit_colsum_half(0)
            elif jt == 15:
                emit_colsum_half(1)
        nc.vector.tensor_tensor(
            out=msum_row, in0=msum_h[:, 0, :], in1=msum_h[:, 1, :], op=Add
        )
        emit_mean_rep()
        emit_scores(1, av_of=0)
        emit_scores(2, av_of=1)
        emit_scores(3, av_of=2)
        for it in range(12, 16):
            emit_av_chunk(it)

    nc.compile()
    return nc


def _fp8_pair(a, npdt):
    """Return (fp8(a), fp8(a - fp8(a))) as numpy arrays of dtype npdt."""
    a = np.asarray(a, np.float32)
    a8 = a.astype(npdt)
    ar = (a - a8.astype(np.float32)).astype(npdt)
    return a8, ar


def _pcn(a):
    """[(c p), n] -> [p, c, n]: the partition-major layout the device DMAs
    contiguously."""
    a = np.asarray(a)
    cn, n = a.shape
    return np.ascontiguousarray(a.reshape(cn // P, P, n).transpose(1, 0, 2))


def make_in_maps(x, event_lengths, Wq, bq, Wk, bk, Wv, bv, n=N, d=D):
    """Host-side sharding + marshaling: one batch element per core."""
    npdt = mybir.dt.np(mybir.dt.float8e4)
    x = np.asarray(x, dtype=np.float32)
    lens = np.asarray(event_lengths).astype(np.int64)
    ws = {}
    for wn, W in (("wq", Wq), ("wk", Wk), ("wv", Wv)):
        wT = np.ascontiguousarray(np.asarray(W, np.float32).T) * WSCALE
        w8, wr = _fp8_pair(wT, npdt)
        ws[wn + "8"], ws[wn + "r"] = _pcn(w8), _pcn(wr)
    # biases enter the PSUM which carries a WSCALE factor; valid/bsel are
    # staged pre-divided by WSCALE so the WSCALE on v' cancels at the output
    bq = np.ascontiguousarray(
        (np.asarray(bq, np.float32) * np.float32(WSCALE)).reshape(D // P, P).T
    )
    bk = np.ascontiguousarray(
        (np.asarray(bk, np.float32) * np.float32(WSCALE)).reshape(D // P, P).T
    )
    idx = np.arange(n)
    valid2d = np.ascontiguousarray(
        (idx.reshape(n // P, P).T[None, :, :] < lens[:, None, None])
    ).astype(np.float32)  # [B, P, nt] : valid2d[b, p, t] = (t*128+p < L_b)
    in_maps = []
    for b in range(x.shape[0]):
        va = valid2d[b]
        x8, xr = _fp8_pair(np.ascontiguousarray(x[b].T), npdt)
        in_maps.append(
            {
                "x8": _pcn(x8),
                "xr": _pcn(xr),
                **ws,
                "bq": bq,
                "bk": bk,
                "maskb": (1.0 - va) * MASK_VAL,
                "avalid": va / np.float32(WSCALE),
                "bsel": (1.0 - va) / np.float32(n * WSCALE),
            }
        )
    return in_maps


_NC_CACHE = {}


def kernel(x, event_lengths, Wq, bq, Wk, bk, Wv, bv):
    from concourse.bass_utils import run_bass_kernel_spmd

    if "nc" not in _NC_CACHE:
        _NC_CACHE["nc"] = build_attention_nc()
    nc = _NC_CACHE["nc"]
    in_maps = make_in_maps(x, event_lengths, Wq, bq, Wk, bk, Wv, bv)
    res = run_bass_kernel_spmd(nc, in_maps, core_ids=list(range(B)))
    out = np.stack([np.asarray(r["out"], np.float32) for r in res.results], axis=0)
    return out



# revision 5
# speedup vs baseline: 1.6536x; 1.6536x over previous
"""Single-head attention with per-sample padding masks on 8 Trainium2
NeuronCores — length-aware work rebalancing.

kernel(**inputs) takes the FULL unsharded inputs and returns the FULL
[B, N, D] float32 output.

The per-sample event_lengths are known when kernel() is called, so the
device program is built (and cached) per lengths-tuple.  Valid attention
work scales as (L/N)^2 per sample; instead of one sample per core, the
512-query BLOCKS of all samples are bin-packed across the 8 cores:

  - Each core owns QB = ceil(total_blocks/8) query-block slots and a
    packed key-set of up to KT=16 key tiles (128 rows each).  A core's
    key-set concatenates the key tiles of every sample whose blocks it
    hosts; the per-(key-tile, block) mask bias (0 valid / -1e9) closes
    cross-sample and padded-key positions, so packing costs nothing.
  - Padded-query rows (i >= L) need softmax over an all-masked row =
    mean of v over ALL N rows; that is colsum(x) @ Wv.T / N + bv,
    computed EXACTLY on the host and scattered in during assembly, so
    the device mean path (colsum matmuls, mean replication, output
    blend) disappears.
  - Idle slots (capacity rounding) attend tile 0 of the core's key-set
    with an open mask so rowsum >= 1 (finite garbage, discarded).

Device numerics are unchanged from the tuned baseline (8.9e-3 rel err):
residual-compensated fp8 DoubleRow matmuls, W' = WSCALE*W staged as fp8
pairs, q'/k' quantized on-device to fp8 pairs, scores 2-term
(k8q8 + k8qr), exp via ACT with the mask riding the bias, AV 3-term
(a8v8 + a8vr + arv8) with a 2-term rowsum, out = AV * 1/rowsum.
"""

import math
import sys
from contextlib import ExitStack

import numpy as np

sys.path.insert(0, "/opt/trn_rl_repo")

import concourse.mybir as mybir  # noqa: E402
import concourse.tile as tile  # noqa: E402
from concourse import bacc  # noqa: E402

P = 128
B, N, D = 8, 2048, 512
FB = 512  # psum free-dim block (one bank) = query-block width
KT = 16  # key tiles per core (packed key-set capacity)
MASK_VAL = -1.0e9
# Weights pre-scaled into fp8 normal range.  32 (not 64): q' = WSCALE*q must
# stay below fp8 e4m3 max 240.
WSCALE = 32.0


def plan_assignment(lens, n=N):
    """Bin-pack 512-query blocks onto 8 cores.

    Returns (QB, slots) where slots[c] is a list of length QB of either
    (sample, block_idx) or None (idle), plus keysets[c]: ordered list of
    (sample, kt_count) giving the packed key-tile layout of core c.
    """
    lens = [int(l) for l in lens]
    nb_s = [max(1, math.ceil(l / FB)) for l in lens]
    kt_s = [max(1, math.ceil(l / P)) for l in lens]
    total = sum(nb_s)
    QB = max(1, math.ceil(total / 8))
    while True:
        order = sorted(range(len(lens)), key=lambda b: -kt_s[b])
        slots = [[] for _ in range(8)]
        keysets = [[] for _ in range(8)]
        keyused = [0] * 8
        ok = True
        for b in order:
            remaining = nb_s[b]
            while remaining > 0:
                best, best_cost = None, None
                for c in range(8):
                    if len(slots[c]) >= QB:
                        continue
                    add = 0 if any(s == b for s, _ in keysets[c]) else kt_s[b]
                    if keyused[c] + add > KT:
                        continue
                    space = QB - len(slots[c])
                    # best-fit: least keyset growth, then tightest slot fit
                    cost = (add, space)
                    if best is None or cost < best_cost:
                        best, best_cost = c, cost
                if best is None:
                    ok = False
                    break
                c = best
                if not any(s == b for s, _ in keysets[c]):
                    keysets[c].append((b, kt_s[b]))
                    keyused[c] += kt_s[b]
                take = min(remaining, QB - len(slots[c]))
                start = nb_s[b] - remaining
                for j in range(start, start + take):
                    slots[c].append((b, j))
                remaining -= take
            if not ok:
                break
        if ok:
            for c in range(8):
                while len(slots[c]) < QB:
                    slots[c].append(None)
            return QB, slots, keysets
        QB += 1


def build_attention_nc(qb, n=N, d=D, debug=False):
    """Build the one-core Bass program for QB query blocks x KT key tiles."""
    f32 = mybir.dt.float32
    bf16 = mybir.dt.bfloat16
    fp8 = mybir.dt.float8e4
    DR = mybir.MatmulPerfMode.DoubleRow
    ec_n = d // P  # embedding chunks (contraction over E and D)
    nq = qb * FB  # query columns on this core
    nk = KT * P  # packed key rows on this core
    s = 1.0 / math.sqrt(d)

    nc = bacc.Bacc(None, target_bir_lowering=False, debug=debug)

    # inputs staged host-side in [P, chunk, cols] layout: contiguous per
    # partition
    xq8_d = nc.declare_dram_parameter("xq8", [P, ec_n, nq], fp8, isOutput=False)
    xqr_d = nc.declare_dram_parameter("xqr", [P, ec_n, nq], fp8, isOutput=False)
    xk8_d = nc.declare_dram_parameter("xk8", [P, ec_n, nk], fp8, isOutput=False)
    xkr_d = nc.declare_dram_parameter("xkr", [P, ec_n, nk], fp8, isOutput=False)
    w8_ds, wr_ds = {}, {}
    for wn in ("wq", "wk", "wv"):
        w8_ds[wn] = nc.declare_dram_parameter(
            wn + "8", [P, ec_n, d], fp8, isOutput=False
        )
        wr_ds[wn] = nc.declare_dram_parameter(
            wn + "r", [P, ec_n, d], fp8, isOutput=False
        )
    bq_d = nc.declare_dram_parameter("bq", [P, ec_n], f32, isOutput=False)
    bk_d = nc.declare_dram_parameter("bk", [P, ec_n], f32, isOutput=False)
    maskb_d = nc.declare_dram_parameter("maskb", [P, KT, qb], f32, isOutput=False)
    out_d = nc.declare_dram_parameter("out", [nq, d], f32, isOutput=True)

    Ident = mybir.ActivationFunctionType.Identity
    Exp = mybir.ActivationFunctionType.Exp
    Add = mybir.AluOpType.add
    Mult = mybir.AluOpType.mult
    Sub = mybir.AluOpType.subtract

    with tile.TileContext(nc) as tc, ExitStack() as ctx:
        const = ctx.enter_context(tc.tile_pool(name="const", bufs=1))
        big = ctx.enter_context(tc.tile_pool(name="big", bufs=1))
        work = ctx.enter_context(tc.tile_pool(name="work", bufs=8))
        small = ctx.enter_context(tc.tile_pool(name="small", bufs=4))
        psum_s = ctx.enter_context(tc.tile_pool(name="psum_s", bufs=4, space="PSUM"))
        psum_av = ctx.enter_context(tc.tile_pool(name="psum_av", bufs=3, space="PSUM"))
        psum_rs = ctx.enter_context(tc.tile_pool(name="psum_rs", bufs=1, space="PSUM"))

        # ---- constants / parameters into SBUF ----
        bq_sb = const.tile([P, ec_n], f32)
        bk_sb = const.tile([P, ec_n], f32)
        maskb_sb = const.tile([P, KT, qb], f32)
        zero_d = const.tile([P, d], f32)
        nc.vector.memset(zero_d, 0.0)

        xq8_sb = big.tile([P, ec_n, nq], fp8)
        xqr_sb = big.tile([P, ec_n, nq], fp8)
        xk8_sb = big.tile([P, ec_n, nk], fp8)
        xkr_sb = big.tile([P, ec_n, nk], fp8)
        w8_sb = {wn: big.tile([P, ec_n, d], fp8, name=wn + "8_sb") for wn in ("wq", "wk", "wv")}
        wr_sb = {wn: big.tile([P, ec_n, d], fp8, name=wn + "r_sb") for wn in ("wq", "wk", "wv")}
        # coalesced contiguous DMAs spread over the 3 DMA-capable queues;
        # K-side first (K-proj opens the pipeline), then Q-side, then V.
        nc.scalar.dma_start(out=w8_sb["wk"], in_=w8_ds["wk"][:, :, :])
        nc.scalar.dma_start(out=wr_sb["wk"], in_=wr_ds["wk"][:, :, :])
        nkb = nk // FB
        for ib in range(nkb):
            if ib == 0:
                # first block split in dc halves so the opening DoubleRow
                # pair (dc 0..1) lands earlier
                for h in range(2):
                    nc.sync.dma_start(
                        out=xk8_sb[:, 2 * h : 2 * h + 2, 0:FB],
                        in_=xk8_d[:, 2 * h : 2 * h + 2, 0:FB],
                    )
                    nc.gpsimd.dma_start(
                        out=xkr_sb[:, 2 * h : 2 * h + 2, 0:FB],
                        in_=xkr_d[:, 2 * h : 2 * h + 2, 0:FB],
                    )
                nc.gpsimd.dma_start(out=bq_sb, in_=bq_d[:, :])
                nc.gpsimd.dma_start(out=bk_sb, in_=bk_d[:, :])
            else:
                nc.sync.dma_start(
                    out=xk8_sb[:, :, ib * FB : (ib + 1) * FB],
                    in_=xk8_d[:, :, ib * FB : (ib + 1) * FB],
                )
                nc.gpsimd.dma_start(
                    out=xkr_sb[:, :, ib * FB : (ib + 1) * FB],
                    in_=xkr_d[:, :, ib * FB : (ib + 1) * FB],
                )
            if ib == 0:
                nc.sync.dma_start(out=w8_sb["wq"], in_=w8_ds["wq"][:, :, :])
                nc.sync.dma_start(out=wr_sb["wq"], in_=wr_ds["wq"][:, :, :])
                nc.sync.dma_start(out=xq8_sb[:, :, 0:FB], in_=xq8_d[:, :, 0:FB])
                nc.gpsimd.dma_start(out=xqr_sb[:, :, 0:FB], in_=xqr_d[:, :, 0:FB])
            elif ib == 1:
                nc.scalar.dma_start(out=w8_sb["wv"], in_=w8_ds["wv"][:, :, :])
                nc.scalar.dma_start(out=wr_sb["wv"], in_=wr_ds["wv"][:, :, :])
                nc.gpsimd.dma_start(out=maskb_sb, in_=maskb_d[:, :, :])
        for ib in range(1, qb):
            nc.sync.dma_start(
                out=xq8_sb[:, :, ib * FB : (ib + 1) * FB],
                in_=xq8_d[:, :, ib * FB : (ib + 1) * FB],
            )
            nc.gpsimd.dma_start(
                out=xqr_sb[:, :, ib * FB : (ib + 1) * FB],
                in_=xqr_d[:, :, ib * FB : (ib + 1) * FB],
            )

        q8_sb = big.tile([P, ec_n, nq], fp8)
        qr_sb = big.tile([P, ec_n, nq], fp8)
        k8_sb = big.tile([P, ec_n, nk], fp8)
        kr_sb = big.tile([P, ec_n, nk], fp8)
        v_sb = big.tile([P, KT, d], bf16)
        v8_sb = big.tile([P, KT, d], fp8)
        vr_sb = big.tile([P, KT, d], fp8)
        a8_sb = big.tile([P, KT, nq], fp8)
        ar_sb = big.tile([P, KT, nq], fp8)
        # rowsum is taken against WSCALE (not 1) so 1/rowsum' also cancels
        # the WSCALE carried by v' in the AV numerator: num·W / (den·W)
        ones2_col = const.tile([P, 2, 1], fp8)
        nc.vector.memset(ones2_col, WSCALE)
        ones_row = const.tile([1, P], bf16)
        nc.vector.memset(ones_row, 1.0)
        # p-state warm-up: keep the PE busy during the initial DMA window so
        # real work starts at full clock
        warm = const.tile([1, FB], bf16)
        nc.vector.memset(warm, 1.0)
        for _ in range(6):
            pd = psum_rs.tile([P, FB], f32, tag="prs")
            nc.tensor.matmul(pd, lhsT=ones_row, rhs=warm, start=True, stop=True)

        def mm3(ps, a8, ar, b8, br, asl, bsl, drop_ar=False):
            """psum += a@b as a8@b8 + a8@br + ar@b8 (fp8 DoubleRow terms).
            drop_ar: omit ar@b8 (scores only; measured +8.6e-3 in quadrature,
            total 8.9e-3 vs the 2e-2 gate)."""
            terms = [(a8, b8), (a8, br)]
            if not drop_ar:
                terms.append((ar, b8))
            nmm = len(terms) * (ec_n // 2)
            i = 0
            for ta, tb in terms:
                for ecp in range(0, ec_n, 2):
                    nc.tensor.matmul(
                        ps,
                        lhsT=asl(ta, ecp),
                        rhs=bsl(tb, ecp),
                        start=(i == 0),
                        stop=(i == nmm - 1),
                        perf_mode=DR,
                    )
                    i += 1

        # ---- projections; W-residuals are NOT droppable (uniform weights
        # quantize 1.6x worse; measured 1.7e-2 alone) ----
        def emit_proj_unit(wn, b_sb, o8, orr, x8, xr, ib):
            for ec in range(ec_n):
                if ec % 2 == 0:
                    ps = psum_s.tile([P, FB], f32, tag="ps", name="ps_qk")
                else:
                    ps = psum_av.tile([P, FB], f32, tag="pav", name="ps_qk")
                mm3(
                    ps,
                    w8_sb[wn],
                    wr_sb[wn],
                    x8,
                    xr,
                    lambda t, ecp, ec=ec: t[:, ecp : ecp + 2, ec * P : (ec + 1) * P],
                    lambda t, ecp, ib=ib: t[:, ecp : ecp + 2, ib * FB : (ib + 1) * FB],
                )
                sl = (slice(None), ec, slice(ib * FB, (ib + 1) * FB))
                nc.scalar.activation(
                    o8[sl], ps, Ident, bias=b_sb[:, ec : ec + 1], scale=1.0
                )
                nc.vector.scalar_tensor_tensor(
                    out=orr[sl],
                    in0=ps,
                    scalar=b_sb[:, ec : ec + 1],
                    in1=o8[sl],
                    op0=Add,
                    op1=Sub,
                )

        def emit_vproj(jt):
            ps = psum_s.tile([P, d], f32, tag="ps")
            mm3(
                ps,
                xk8_sb,
                xkr_sb,
                w8_sb["wv"],
                wr_sb["wv"],
                lambda t, ecp, jt=jt: t[:, ecp : ecp + 2, jt * P : (jt + 1) * P],
                lambda t, ecp: t[:, ecp : ecp + 2, :],
            )
            nc.scalar.activation(v_sb[:, jt, :], ps, Ident, bias=0.0, scale=1.0)
            nc.gpsimd.tensor_copy(v8_sb[:, jt, :], v_sb[:, jt, :])
            nc.vector.tensor_tensor(
                out=vr_sb[:, jt, :], in0=v_sb[:, jt, :], in1=v8_sb[:, jt, :], op=Sub
            )

        # ---- attention ----
        def emit_scores_unit(ib, jt):
            ps = psum_s.tile([P, FB], f32, tag="ps")
            mm3(
                ps,
                k8_sb,
                kr_sb,
                q8_sb,
                qr_sb,
                lambda t, ecp, jt=jt: t[:, ecp : ecp + 2, jt * P : (jt + 1) * P],
                lambda t, ecp, ib=ib: t[:, ecp : ecp + 2, ib * FB : (ib + 1) * FB],
                drop_ar=True,
            )
            ab = work.tile([P, FB], bf16)
            nc.scalar.activation(
                ab,
                ps,
                Exp,
                bias=maskb_sb[:, jt, ib : ib + 1],
                scale=s / (WSCALE * WSCALE),
            )
            sl = (slice(None), jt, slice(ib * FB, (ib + 1) * FB))
            nc.gpsimd.tensor_copy(a8_sb[sl], ab)
            nc.vector.tensor_tensor(out=ar_sb[sl], in0=ab, in1=a8_sb[sl], op=Sub)

        def emit_scores(ib, av_of=None):
            # av_of: interleave the 4 AV chunks of that (older, data-ready)
            # block between this block's scores units
            for jt in range(KT):
                if av_of is not None and jt >= 4 and (jt - 4) % 4 == 0:
                    emit_av_chunk(av_of * 4 + (jt - 4) // 4)
                emit_scores_unit(ib, jt)
            if av_of is not None:
                emit_av_chunk(av_of * 4 + 3)

        def emit_av_chunk(it):
            pav = psum_av.tile([P, d], f32, tag="pav")
            prs = psum_rs.tile([P, 1], f32, tag="prs")
            av_terms = [(a8_sb, v8_sb), (a8_sb, vr_sb), (ar_sb, v8_sb)]
            nmm = len(av_terms) * (KT // 2)
            i = 0
            for ta, tv in av_terms:
                for jtp in range(0, KT, 2):
                    nc.tensor.matmul(
                        pav,
                        lhsT=ta[:, jtp : jtp + 2, it * P : (it + 1) * P],
                        rhs=tv[:, jtp : jtp + 2, :],
                        start=(i == 0),
                        stop=(i == nmm - 1),
                        perf_mode=DR,
                    )
                    i += 1
            i = 0
            for ta in (a8_sb, ar_sb):
                for jtp in range(0, KT, 2):
                    nc.tensor.matmul(
                        prs,
                        lhsT=ta[:, jtp : jtp + 2, it * P : (it + 1) * P],
                        rhs=ones2_col,
                        start=(i == 0),
                        stop=(i == 2 * (KT // 2) - 1),
                        perf_mode=DR,
                    )
                    i += 1
            rinv = small.tile([P, 1], f32)
            nc.vector.reciprocal(rinv, prs)
            outt = work.tile([P, d], f32)
            nc.vector.scalar_tensor_tensor(
                out=outt,
                in0=pav,
                scalar=rinv,
                in1=zero_d,
                op0=Mult,
                op1=Add,
            )
            nc.sync.dma_start(out=out_d[it * P : (it + 1) * P, :], in_=outt)

        # software-pipelined emission:
        #  - K projection first (scores(0) needs it), Q and V interleaved;
        #  - scores(0) slotted in as soon as k-columns + q-block 0 exist;
        #  - AV chunks of block ib interleave into scores(ib+1).
        nkb = nk // FB
        for ib in range(nkb):
            emit_proj_unit("wk", bk_sb, k8_sb, kr_sb, xk8_sb, xkr_sb, ib)
            if ib < qb:
                emit_proj_unit("wq", bq_sb, q8_sb, qr_sb, xq8_sb, xqr_sb, ib)
            for jt in range(ib * 4, ib * 4 + 4):
                if jt < 12:
                    emit_vproj(jt)
        for jt in range(KT):
            if jt in (3, 6, 9, 12):
                emit_vproj(jt // 3 + 11)
            emit_scores_unit(0, jt)
        for ib in range(1, qb):
            emit_scores(ib, av_of=ib - 1)
        for it in range((qb - 1) * 4, qb * 4):
            emit_av_chunk(it)

    nc.compile()
    return nc


def _fp8_pair(a, npdt):
    a = np.asarray(a, np.float32)
    a8 = a.astype(npdt)
    ar = (a - a8.astype(np.float32)).astype(npdt)
    return a8, ar


def _pcn(a):
    """[(c p), n] -> [p, c, n] partition-major layout."""
    a = np.asarray(a)
    cn, n = a.shape
    return np.ascontiguousarray(a.reshape(cn // P, P, n).transpose(1, 0, 2))


def make_in_maps(x, event_lengths, Wq, bq, Wk, bk, Wv, bv):
    """Host-side planning + marshaling. Returns (in_maps, plan)."""
    npdt = mybir.dt.np(mybir.dt.float8e4)
    x = np.asarray(x, dtype=np.float32)
    lens = np.asarray(event_lengths).astype(np.int64)
    qb, slots, keysets = plan_assignment(lens)
    nq = qb * FB
    nk = KT * P
    ws = {}
    for wn, W in (("wq", Wq), ("wk", Wk), ("wv", Wv)):
        wT = np.ascontiguousarray(np.asarray(W, np.float32).T) * WSCALE
        w8, wr = _fp8_pair(wT, npdt)
        ws[wn + "8"], ws[wn + "r"] = _pcn(w8), _pcn(wr)
    bq_m = np.ascontiguousarray(
        (np.asarray(bq, np.float32) * np.float32(WSCALE)).reshape(D // P, P).T
    )
    bk_m = np.ascontiguousarray(
        (np.asarray(bk, np.float32) * np.float32(WSCALE)).reshape(D // P, P).T
    )
    xT = {b: np.ascontiguousarray(x[b].T) for b in range(x.shape[0])}
    in_maps = []
    for c in range(8):
        # packed key-side x: concat keyset samples' first kt*128 seq cols
        xk = np.zeros((D, nk), np.float32)
        ktpos = {}  # sample -> starting key tile
        pos = 0
        for b, kt in keysets[c]:
            ktpos[b] = pos
            xk[:, pos * P : pos * P + kt * P] = xT[b][:, : kt * P]
            pos += kt
        # query-side x: per slot, that sample's block columns
        xq = np.zeros((D, nq), np.float32)
        maskb = np.full((P, KT, qb), MASK_VAL, np.float32)
        for j, slot in enumerate(slots[c]):
            if slot is None:
                maskb[:, 0, j] = 0.0  # keep rowsum >= 1; output discarded
                continue
            b, blk = slot
            xq[:, j * FB : (j + 1) * FB] = xT[b][:, blk * FB : (blk + 1) * FB]
            base = ktpos[b]
            ktn = dict(keysets[c])[b]
            L = int(lens[b])
            for t in range(ktn):
                valid = (t * P + np.arange(P)) < L
                maskb[:, base + t, j] = np.where(valid, 0.0, MASK_VAL)
        xk8, xkr = _fp8_pair(xk, npdt)
        xq8, xqr = _fp8_pair(xq, npdt)
        in_maps.append(
            {
                "xq8": _pcn(xq8),
                "xqr": _pcn(xqr),
                "xk8": _pcn(xk8),
                "xkr": _pcn(xkr),
                **ws,
                "bq": bq_m,
                "bk": bk_m,
                "maskb": maskb,
            }
        )
    return in_maps, (qb, slots)


_NC_CACHE = {}


def kernel(x, event_lengths, Wq, bq, Wk, bk, Wv, bv):
    from concourse.bass_utils import run_bass_kernel_spmd

    x = np.asarray(x, np.float32)
    lens = np.asarray(event_lengths).astype(np.int64)
    in_maps, (qb, slots) = make_in_maps(x, lens, Wq, bq, Wk, bk, Wv, bv)
    if qb not in _NC_CACHE:
        _NC_CACHE[qb] = build_attention_nc(qb)
    nc = _NC_CACHE[qb]
    res = run_bass_kernel_spmd(nc, in_maps, core_ids=list(range(8)))
    # host assembly: scatter core blocks, then fill padded-query rows with
    # the exact mean of v over all N rows (softmax over an all-masked row).
    Wv32 = np.asarray(Wv, np.float32)
    bv32 = np.asarray(bv, np.float32)
    mean_v = (x.sum(axis=1) @ Wv32.T) / np.float32(N) + bv32  # [B, D]
    out = np.empty((B, N, D), np.float32)
    for c in range(8):
        co = np.asarray(res.results[c]["out"], np.float32)
        for j, slot in enumerate(slots[c]):
            if slot is None:
                continue
            b, blk = slot
            out[b, blk * FB : (blk + 1) * FB, :] = co[j * FB : (j + 1) * FB, :]
    for b in range(B):
        L = int(lens[b])
        if L < N:
            out[b, L:, :] = mean_v[b][None, :]
    return out
